# revision 77
# baseline (speedup 1.0000x reference)
"""Trainium2 Bass kernel for nn_Attention_47124381171831.

Dense transformer block: 1x1-conv QKV projections (+BN eval), 8-head
attention over 1024 positions with a gathered relative-position bias,
ReLU, 1x1-conv output projection (+BN eval).

Sharding: pure data-parallel over batch. B=16 -> 2 batches per core on
8 cores, zero collectives. All weights / tables replicated.

Per-core design (single software-pipelined stream, ACT-bound):
  - positional bias applied MULTIPLICATIVELY:
      softmax(qk*s + bias/s) = normalize(exp(qk*s) * E),
      E = exp(pos_tab/s)[pos_idx] (host-expanded per head, streamed in)
  - exp on ACT is the hard floor (~134us busy): ACT has no 16-bit
    speedup, so every other engine is scheduled underneath it.
  - one shared PSUM ring ([128,1024] f32 x2) carries dots / qkv / wo
    chains; two [128,1024] av tiles alternate per head-batch.
  - dots in fp8e4 DoubleRow mode (0.5 cyc/col): q/k stored [128,2,N]
    with the r=1 plane zeroed, so no partition shuffle is needed
    (plain DoubleRow on HW contracts (p, r) pairs; verified by probe).
  - attn@v via augmented v (even head: [v|1] -> Z row 64; odd head:
    [0x32|1|0x31|v] -> Z row 32, O rows 64:128) so the drain is
    partition-aligned with the ro halves; av matmuls trail their w2
    by several pairs (deep ring) to decouple PE from the exp chain.
  - drain: DVE recip of the Z row (bf16), 1/Z broadcast across
    partitions via a DRAM bounce (store row + stride-0 re-load; gpsimd
    partition_broadcast is runtime-broken on HW), DVE psum-mult, then
    fused per-partition v-bias + relu (softmax weights sum to 1, so
    attn@(v+b) = attn@v + b — the v projection copies stay biasless).
  - b1 lags b0 by one h-pair so early qkv fillers spread over 4 rows;
    wo rides the av ring / tail with the h7 contraction rows split out.
"""

import sys
from collections import deque

import numpy as np

sys.path.insert(0, "/opt/trn_rl_repo")

import ml_dtypes  # noqa: E402

BF16 = ml_dtypes.bfloat16

B, C, F, H, DK, DV = 16, 256, 32, 8, 32, 64
N = F * F  # 1024
EPS = 1e-5
SCALE = DK ** -0.5
NCORES = 8
BL = B // NCORES  # batches per core

USE_FP8_DOTS = True
ADDITIVE_BIAS = False

_CACHE = {}


def _build_bass():
    import concourse.bacc as bacc
    import concourse.tile as tile
    from concourse import mybir

    f32 = mybir.dt.float32
    bf16 = mybir.dt.bfloat16
    fp8 = mybir.dt.float8e4
    AF = mybir.ActivationFunctionType
    ALU = mybir.AluOpType
    PM = mybir.MatmulPerfMode

    nc = bacc.Bacc("TRN2", target_bir_lowering=False)

    x_d = nc.dram_tensor("x", [BL, 2, 128, N], bf16, kind="ExternalInput")
    wqk_d = nc.dram_tensor("wqk", [2, 128, 512], bf16, kind="ExternalInput")
    wv_d = nc.dram_tensor("wv", [2, 128, 512], bf16, kind="ExternalInput")
    wo_d = nc.dram_tensor("wo", [4, 128, 256], bf16, kind="ExternalInput")
    qkb_d = nc.dram_tensor("qkb", [128, 4], f32, kind="ExternalInput")
    vbp_d = nc.dram_tensor("vbp", [128, 4], f32, kind="ExternalInput")
    wobp_d = nc.dram_tensor("wobp", [128, 2], f32, kind="ExternalInput")
    e_d = nc.dram_tensor("etab", [H, 8, 128, N], bf16, kind="ExternalInput")
    b8_d = nc.dram_tensor("b8tab", [H, 2, 128, 2, 2, 2, 512], fp8,
                          kind="ExternalInput")
    id2_d = nc.dram_tensor("id2", [128, 2, 128], fp8, kind="ExternalInput")
    vaug_d = nc.dram_tensor("vaug", [128, 772], bf16, kind="ExternalInput")
    qkz_d = nc.dram_tensor("qkz", [128, N], fp8, kind="ExternalInput")
    zr_d = nc.dram_tensor("zrscratch", [16, N], bf16, kind="Internal")
    out_d = nc.dram_tensor("out", [BL, 2, 128, N], bf16,
                           kind="ExternalOutput")

    def is_add(h, jp):
        # tiles with bias applied additively in PSUM (two-term fp8);
        # jp-odd so the first tile of the kernel is the (cheaper-to-load)
        # multiplicative path
        return ADDITIVE_BIAS and jp % 2 == 1

    with tile.TileContext(nc) as tc:
        with (
            tc.tile_pool(name="const", bufs=1) as cpool,
            tc.tile_pool(name="acts", bufs=1) as apool,
            tc.tile_pool(name="etile", bufs=8) as epool,
            tc.tile_pool(name="btile", bufs=8) as b8pool,
            tc.tile_pool(name="wexp", bufs=3) as wpool,
            tc.tile_pool(name="w2p", bufs=3) as w2pool,
            tc.tile_pool(name="zrp", bufs=2) as zrpool,
            tc.tile_pool(name="rzbp", bufs=2) as rzbpool,
            tc.tile_pool(name="outs", bufs=2) as opool,
            tc.tile_pool(name="ps", bufs=2, space="PSUM") as spool,
            tc.tile_pool(name="avp", bufs=2, space="PSUM") as avpool,
        ):
            # ---------------- constants / persistent tiles ----------------
            wqk_sb = [cpool.tile([128, 512], bf16, name=f"wqk{i}",
                                 tag=f"wqk{i}") for i in range(2)]
            wv_sb = [cpool.tile([128, 512], bf16, name=f"wv{i}",
                                tag=f"wv{i}") for i in range(2)]
            wo_sb = [cpool.tile([128, 256], bf16, name=f"wo{i}",
                                tag=f"wo{i}") for i in range(4)]
            qkb_sb = cpool.tile([128, 4], f32, name="qkb", tag="qkb")
            vbb_sb = cpool.tile([128, 512], bf16, name="vbb", tag="vbb")
            vbp_sb = cpool.tile([128, 4], f32, name="vbp", tag="vbp")
            wobp_sb = cpool.tile([128, 2], f32, name="wobp", tag="wobp")
            id2_sb = cpool.tile([128, 2, 128], fp8, name="id2", tag="id2")
            ones_col = cpool.tile([128, 64], bf16, name="ones_col",
                                  tag="ones_col")

            x_sb = [[apool.tile([128, N], bf16, name=f"x{b}{ck}",
                                tag=f"x{b}{ck}") for ck in range(2)]
                    for b in range(BL)]
            if USE_FP8_DOTS:
                q_sb = [[apool.tile([128, 2, N], fp8, name=f"q{b}{ct}",
                                    tag=f"q{b}{ct}") for ct in range(2)]
                        for b in range(BL)]
                k_sb = [[apool.tile([128, 2, N], fp8, name=f"k{b}{ct}",
                                    tag=f"k{b}{ct}") for ct in range(2)]
                        for b in range(BL)]
            else:
                q_sb = [[apool.tile([128, N], bf16, name=f"q{b}{ct}",
                                    tag=f"q{b}{ct}") for ct in range(2)]
                        for b in range(BL)]
                k_sb = [[apool.tile([128, N], bf16, name=f"k{b}{ct}",
                                    tag=f"k{b}{ct}") for ct in range(2)]
                        for b in range(BL)]
            # augmented v: even heads he=h//2 at cols [he*65, he*65+65) as
            # [v|1]; odd heads at cols [260+ho*128, +128) as
            # [0(32)|1|0(31)|v] so Z lands on PSUM row 32 and O on
            # partitions 64:128 (partition-aligned with ro halves).
            v_sb = [[apool.tile([128, 772], bf16, name=f"v{b}{jt}",
                                tag=f"v{b}{jt}") for jt in range(8)]
                    for b in range(BL)]
            ro_sb = [[apool.tile([128, N], bf16, name=f"ro{b}{ctk}",
                                 tag=f"ro{b}{ctk}") for ctk in range(4)]
                     for b in range(BL)]

            # ---------------- DMA preloads (priority order) ---------------
            nc.sync.dma_start(wqk_sb[0][:], wqk_d[0])
            nc.sync.dma_start(x_sb[0][0][:], x_d[0, 0])
            nc.sync.dma_start(qkb_sb[:], qkb_d[:])
            nc.sync.dma_start(wqk_sb[1][:], wqk_d[1])
            nc.sync.dma_start(x_sb[0][1][:], x_d[0, 1])
            nc.gpsimd.memset(ones_col[:], 1.0)
            if USE_FP8_DOTS:
                for b in range(BL):
                    for ctx_ in range(2):
                        nc.gpsimd.memset(q_sb[b][ctx_][:, 1, :], 0.0)
                        nc.gpsimd.memset(k_sb[b][ctx_][:, 1, :], 0.0)
            for b in range(BL):
                for jt in range(8):
                    vt = v_sb[b][jt]
                    nc.gpsimd.memset(vt[:, 64:260:65], 1.0)
                    nc.gpsimd.memset(
                        vt[:, 260:772].rearrange("p (g c) -> p g c", c=128)
                        [:, :, 0:64], 0.0)
                    nc.gpsimd.memset(vt[:, 292:772:128], 1.0)

            e_tiles = {}

            def fetch_tab(h, jp):
                if is_add(h, jp):
                    bt = b8pool.tile([128, 2, 2, 2, 512], fp8, name="bt",
                                     tag="bt")
                    nc.sync.dma_start(bt[:], b8_d[h, jp // 2])
                    e_tiles[(h, jp)] = bt
                else:
                    et = epool.tile([128, 2, N], bf16, name="et", tag="et")
                    nc.sync.dma_start(
                        et[:], e_d[h, 2 * jp:2 * jp + 2].rearrange(
                            "t p i -> p t i"))
                    e_tiles[(h, jp)] = et[:].rearrange("p t i -> p (t i)")

            fetch_tab(0, 0)
            nc.sync.dma_start(id2_sb[:], id2_d[:])
            for i in range(2):
                nc.sync.dma_start(wv_sb[i][:], wv_d[i])
            fetch_tab(0, 1)
            for ck in range(2):
                nc.sync.dma_start(x_sb[1][ck][:], x_d[1, ck])
            fetch_tab(0, 2)
            fetch_tab(0, 3)
            nc.sync.dma_start(vbp_sb[:], vbp_d[:])
            for i in range(4):
                nc.sync.dma_start(wo_sb[i][:], wo_d[i])
            nc.sync.dma_start(wobp_sb[:], wobp_d[:])

            # ---------------- chain emitters ------------------------------
            def qk_chain(b, mt):
                # mt 0,1 -> q rows ct=mt; mt 2,3 -> k rows ct=mt-2
                s = spool.tile([128, N], f32, name="qks", tag="s")
                for half in (0, 1):
                    hs = slice(half * 512, (half + 1) * 512)
                    for ck in (0, 1):
                        nc.tensor.matmul(
                            s[:, hs],
                            wqk_sb[ck][:, mt * 128:(mt + 1) * 128],
                            x_sb[b][ck][:, hs],
                            start=(ck == 0), stop=(ck == 1))
                dst = (q_sb if mt < 2 else k_sb)[b][mt % 2]
                dview = dst[:, 0, :] if USE_FP8_DOTS else dst[:]
                nc.vector.tensor_scalar(dview, s[:],
                                        qkb_sb[:, mt:mt + 1], None,
                                        ALU.add)

            def v_pair(b, jp):
                s = spool.tile([128, N], f32, name="vps", tag="s")
                for r in (0, 1):
                    jt = 2 * jp + r
                    for ck in (0, 1):
                        nc.tensor.matmul(
                            s[:, r * 512:(r + 1) * 512],
                            x_sb[b][ck][:, jt * 128:(jt + 1) * 128],
                            wv_sb[ck][:],
                            start=(ck == 0), stop=(ck == 1))
                for r in (0, 1):
                    jt = 2 * jp + r
                    sv = s[:, r * 512:(r + 1) * 512].rearrange(
                        "p (h c) -> p h c", c=64)
                    ev = v_sb[b][jt][:, 0:260].rearrange(
                        "p (h c) -> p h c", c=65)
                    od = v_sb[b][jt][:, 260:772].rearrange(
                        "p (h c) -> p h c", c=128)
                    # v-bias is deferred to the drain (softmax weights sum
                    # to 1), so these are plain copies.
                    nc.vector.tensor_copy(ev[:, :, 0:64], sv[:, 0::2, :])
                    nc.vector.tensor_copy(od[:, :, 64:128], sv[:, 1::2, :])

            def wo_tile(b, mt, s=None, finish=True, ring=None):
                # ring=avpool rides the av PSUM slots (safe mid-stream);
                # default spool is for the tail when the dots ring is idle.
                # The ctk=3 split lets the last-drained rows (h7) be
                # accumulated separately at the very end.
                if s is None:
                    pool, tg = (ring, "av") if ring is not None \
                        else (spool, "s")
                    s = pool.tile([128, N], f32, name="wops", tag=tg)
                    for half in (0, 1):
                        hs = slice(half * 512, (half + 1) * 512)
                        for ctk in range(3):
                            nc.tensor.matmul(
                                s[:, hs],
                                wo_sb[ctk][:, mt * 128:(mt + 1) * 128],
                                ro_sb[b][ctk][:, hs],
                                start=(ctk == 0), stop=False)
                        nc.tensor.matmul(
                            s[:, hs],
                            wo_sb[3][0:64, mt * 128:(mt + 1) * 128],
                            ro_sb[b][3][0:64, hs],
                            start=False, stop=False)
                if not finish:
                    return s
                for half in (0, 1):
                    hs = slice(half * 512, (half + 1) * 512)
                    nc.tensor.matmul(
                        s[:, hs],
                        wo_sb[3][64:128, mt * 128:(mt + 1) * 128],
                        ro_sb[b][3][64:128, hs],
                        start=False, stop=True, tile_position=(64, 0))
                ot = opool.tile([128, N], bf16, name="ot", tag="ot")
                nc.vector.tensor_scalar(ot[:], s[:], wobp_sb[:, mt:mt + 1],
                                        None, ALU.add)
                nc.sync.dma_start(out_d[b, mt], ot[:])
                return s

            def drain(h, b, av_t, last=False):
                hp = h % 2
                ctk = h // 2
                t = 2 * h + b
                zrow = 64 if hp == 0 else 32
                osl = slice(hp * 64, hp * 64 + 64)
                zr = zrpool.tile([128, N], bf16, name="zr", tag="zr")
                with nc.allow_low_precision("1/Z in bf16: ~0.4% rel"):
                    nc.vector.reciprocal(zr[zrow:zrow + 1, :],
                                         av_t[zrow:zrow + 1, :])
                ro = ro_sb[b][ctk]
                if last:
                    # tail: the dots ring is idle — PE-broadcast 1/Z there
                    # to skip the DRAM round-trip latency.
                    nc.vector.tensor_copy(ro[osl, :], av_t[osl, :])
                    rzp = spool.tile([128, N], f32, name="rzp", tag="s")
                    for half in (0, 1):
                        hs = slice(half * 512, (half + 1) * 512)
                        nc.tensor.matmul(
                            rzp[osl, hs], ones_col[zrow:zrow + 1, 0:64],
                            zr[zrow:zrow + 1, hs], start=True, stop=True,
                            tile_position=(zrow, hp * 64))
                    nc.vector.tensor_tensor(ro[osl, :], ro[osl, :],
                                            rzp[osl, :], ALU.mult)
                    nc.vector.tensor_scalar(ro[osl, :], ro[osl, :],
                                            vbp_sb[osl, ctk:ctk + 1], 0.0,
                                            ALU.add, ALU.max)
                    return
                # broadcast 1/Z across partitions via a DRAM bounce (gpsimd
                # partition_broadcast is runtime-broken; DMA loads from DRAM
                # may replicate a row).
                nc.sync.dma_start(zr_d[t:t + 1, :], zr[zrow:zrow + 1, :])
                rzb = rzbpool.tile([128, N], bf16, name="rzb", tag="rzb")
                nc.sync.dma_start(rzb[osl, :],
                                  zr_d[t:t + 1, :].broadcast_to((64, N)))
                # normalize first (av/Z), then the deferred v-bias + relu:
                # softmax weights sum to 1, so attn@( v + b_v ) = attn@v
                # + b_v, applied here per-partition with the fused relu.
                nc.vector.tensor_tensor(ro[osl, :], av_t[osl, :],
                                        rzb[osl, :], ALU.mult)
                nc.vector.tensor_scalar(ro[osl, :], ro[osl, :],
                                        vbp_sb[osl, ctk:ctk + 1], 0.0,
                                        ALU.add, ALU.max)

            def make_av(h, b, jp, w2v, av_t, last=False):
                he, ho = h // 2, h // 2
                if h % 2 == 0:
                    vcols = slice(he * 65, he * 65 + 65)
                    orows = slice(0, 65)
                else:
                    vcols = slice(260 + ho * 128, 260 + (ho + 1) * 128)
                    orows = slice(0, 128)

                def go():
                    for r in (0, 1):
                        jt = 2 * jp + r
                        for half in (0, 1):
                            nc.tensor.matmul(
                                av_t[orows, half * 512:(half + 1) * 512],
                                v_sb[b][jt][:, vcols],
                                w2v[:, r * N + half * 512:
                                    r * N + (half + 1) * 512],
                                start=(jp == 0 and r == 0),
                                stop=(jp == 3 and r == 1))
                    if jp == 3:
                        drain(h, b, av_t, last=last)
                return go

            # ---------------- schedule + fillers --------------------------
            # b1 lags b0 by one h-pair so the early qkv fillers spread over
            # four rows instead of two.
            schedule = []
            for hp2 in range(0, 8, 2):
                schedule += [(hp2, 0), (hp2 + 1, 0), (hp2, 1), (hp2 + 1, 1)]

            fillers = {}

            def add_filler(slot, jp, fn):
                fillers.setdefault((slot, jp), []).append(fn)

            for jp in range(4):
                add_filler(0, jp, (lambda jp=jp: v_pair(0, jp)))
            add_filler(1, 0, lambda: qk_chain(1, 0))
            add_filler(1, 1, lambda: qk_chain(1, 2))
            add_filler(1, 2, (lambda: v_pair(1, 0)))
            add_filler(1, 3, (lambda: v_pair(1, 1)))
            add_filler(2, 0, (lambda: v_pair(1, 2)))
            add_filler(2, 1, (lambda: v_pair(1, 3)))
            add_filler(4, 0, lambda: qk_chain(0, 1))
            add_filler(4, 2, lambda: qk_chain(0, 3))
            add_filler(5, 0, lambda: qk_chain(1, 1))
            add_filler(5, 2, lambda: qk_chain(1, 3))
            add_filler(14, 3, lambda: wo_tile(0, 0, ring=avpool))
            add_filler(15, 1, lambda: wo_tile(0, 1, ring=avpool))

            # ---------------- lead-in -------------------------------------
            # warm-up chain: PE's p-state needs ~3us of continuous busy to
            # reach full clock; burn the DMA-wait time on a throwaway tile.
            warm = spool.tile([128, N], f32, name="warm", tag="s")
            for i in range(10):
                nc.tensor.matmul(
                    warm[0:64, 0:512], ones_col[0:1, 0:64],
                    wqk_sb[0][0:1, 0:512], start=True, stop=True)
            qk_chain(0, 0)
            qk_chain(0, 2)

            # ---------------- main attention stream -----------------------
            av_q = deque()
            for slot, (h, b) in enumerate(schedule):
                ct = h // 4
                rr = (h % 4) * 32
                if True:
                    t = 2 * h + b
                    av_t = avpool.tile([128, N], f32, name="avt", tag="av")
                    w = None
                    for jp in range(4):
                        additive = is_add(h, jp)
                        if additive:
                            w2 = w2pool.tile([128, 2 * N], bf16, name="w2t",
                                             tag="w2")
                        for r in (0, 1):
                            jt = 2 * jp + r
                            s = spool.tile([128, N], f32, name="dots",
                                           tag="s")
                            for half in (0, 1):
                                hs = slice(half * 512, (half + 1) * 512)
                                if USE_FP8_DOTS:
                                    nc.tensor.matmul(
                                        s[:, hs],
                                        k_sb[b][ct][rr:rr + 32, :,
                                                    jt * 128:(jt + 1) * 128],
                                        q_sb[b][ct][rr:rr + 32, :, hs],
                                        start=True, stop=not additive,
                                        perf_mode=PM.DoubleRow,
                                        tile_position=(rr, 0))
                                else:
                                    nc.tensor.matmul(
                                        s[:, hs],
                                        k_sb[b][ct][rr:rr + 32,
                                                    jt * 128:(jt + 1) * 128],
                                        q_sb[b][ct][rr:rr + 32, hs],
                                        start=True, stop=not additive,
                                        tile_position=(rr, 0))
                                if additive:
                                    bt = e_tiles[(h, jp)]
                                    nc.tensor.matmul(
                                        s[:, hs], id2_sb[:],
                                        bt[:, r, half, :, :],
                                        start=False, stop=True,
                                        perf_mode=PM.DoubleRow)
                            if additive:
                                nc.scalar.activation(
                                    w2[:, r * N:(r + 1) * N], s[:], AF.Exp)
                            else:
                                if r == 0:
                                    w = wpool.tile([128, 2 * N], bf16,
                                                   name="w", tag="w")
                                nc.scalar.activation(w[:, r * N:(r + 1) * N],
                                                     s[:], AF.Exp)
                        if not additive:
                            w2 = w2pool.tile([128, 2 * N], bf16, name="w2t",
                                             tag="w2")
                            ev = e_tiles[(h, jp)] if b == 0 \
                                else e_tiles.pop((h, jp))
                            nc.vector.tensor_mul(w2[:], w[:], ev)
                        elif b == 1:
                            e_tiles.pop((h, jp))
                        av_q.append(make_av(h, b, jp, w2[:], av_t,
                                            last=(slot == 15)))
                        if slot == 0:
                            fetch_tab(1, jp)
                        elif b == 1 and h + 2 < 8:
                            fetch_tab(h + 2, jp)
                        if len(av_q) >= 2:
                            av_q.popleft()()
                        for fn in fillers.pop((slot, jp), []):
                            fn()
            while av_q:
                av_q.popleft()()
            # tail: b1's wo chains overlap the final drain; only the h7
            # rows of the ctk=3 tile wait on the last norm.
            s10 = wo_tile(1, 0, finish=False)
            s11 = wo_tile(1, 1, finish=False)
            wo_tile(1, 0, s=s10)
            wo_tile(1, 1, s=s11)

    nc.compile()
    return nc


def _prep_host(inputs):
    """Fold BN into weights, expand the bias table, build per-core maps."""
    x = np.asarray(inputs["x"], np.float32).reshape(B, C, N)
    Wq = np.asarray(inputs["Wq"], np.float32)
    Wk = np.asarray(inputs["Wk"], np.float32)
    Wv = np.asarray(inputs["Wv"], np.float32)
    Wo = np.asarray(inputs["Wo"], np.float32)
    bo = np.asarray(inputs["bo"], np.float32)
    pos_tab = np.asarray(inputs["pos_tab"], np.float32)
    pos_idx = np.asarray(inputs["pos_idx"])

    def fold(W, g, b_, m, v, gain=1.0):
        s = (np.asarray(g, np.float32)
             / np.sqrt(np.asarray(v, np.float32) + EPS))
        return W * (gain * s)[:, None], gain * (
            np.asarray(b_, np.float32) - np.asarray(m, np.float32) * s)

    ss = SCALE ** 0.5
    Wqf, bqf = fold(Wq, inputs["q_g"], inputs["q_b"], inputs["q_m"],
                    inputs["q_v"], ss)
    Wkf, bkf = fold(Wk, inputs["k_g"], inputs["k_b"], inputs["k_m"],
                    inputs["k_v"], ss)
    Wvf, bvf = fold(Wv, inputs["v_g"], inputs["v_b"], inputs["v_m"],
                    inputs["v_v"], 1.0)
    s_o = (np.asarray(inputs["o_g"], np.float32)
           / np.sqrt(np.asarray(inputs["o_v"], np.float32) + EPS))
    Wof = Wo * s_o[:, None]
    bof = (bo - np.asarray(inputs["o_m"], np.float32)) * s_o \
        + np.asarray(inputs["o_b"], np.float32)

    wqk = np.concatenate([Wqf.T, Wkf.T], axis=1)          # (256, 512)
    wv = np.ascontiguousarray(Wvf.T)                      # (256, 512)
    wo = np.ascontiguousarray(Wof.T)                      # (512, 256)
    qkb = np.ascontiguousarray(
        np.concatenate([bqf, bkf]).reshape(4, 128).T)     # (128, 4)
    vbp = np.ascontiguousarray(bvf.reshape(4, 128).T)     # (128, 4)
    wobp = np.ascontiguousarray(bof.reshape(2, 128).T)    # (128, 2)

    import ml_dtypes as mld
    F8 = mld.float8_e4m3

    etab = np.exp(pos_tab / SCALE).astype(np.float32)     # (N, H)
    e_full = etab[pos_idx.T, :]                           # (j, i, H)
    e_full = np.ascontiguousarray(e_full.transpose(2, 0, 1)).astype(BF16)

    # additive-bias table: two-term fp8 decomposition of bias/s
    b_full = (pos_tab / SCALE).astype(np.float32)[pos_idx.T, :]
    b_full = b_full.transpose(2, 0, 1)                    # (H, j, i)
    b0 = b_full.astype(F8)
    b1 = (b_full - b0.astype(np.float32)).astype(F8)
    # (h, jp, jtl, p, half, ii) -> (h, jp, p, jtl, half, term, ii); keep
    # the additive jps {1, 3}. Fully contiguous per (jtl, half) so the
    # DoubleRow ifmap AP is a flat 1KB run per partition.
    b8 = np.stack([b0.reshape(H, 4, 2, 128, 2, 512),
                   b1.reshape(H, 4, 2, 128, 2, 512)], axis=5)
    b8 = np.ascontiguousarray(
        b8.transpose(0, 1, 3, 2, 4, 5, 6)[:, 1::2])   # (H,2,128,2,2,2,512)

    id2 = np.zeros((128, 2, 128), np.float32)
    id2[np.arange(128), :, np.arange(128)] = 1.0

    vaug = np.zeros((128, 772), np.float32)
    vaug[:, 64:260:65] = 1.0
    vaug[:, 292:772:128] = 1.0

    common = dict(
        wqk=np.ascontiguousarray(wqk.reshape(2, 128, 512)).astype(BF16),
        wv=wv.reshape(2, 128, 512).astype(BF16),
        wo=wo.reshape(4, 128, 256).astype(BF16),
        qkb=qkb.astype(np.float32),
        vbp=vbp.astype(np.float32),
        wobp=wobp.astype(np.float32),
        etab=e_full.reshape(H, 8, 128, N),
        b8tab=b8,
        id2=id2.astype(F8),
        vaug=vaug.astype(BF16),
        qkz=np.zeros((128, N), F8),
    )
    in_maps = []
    for c in range(NCORES):
        xs = x[c * BL:(c + 1) * BL].reshape(BL, 2, 128, N).astype(BF16)
        m = dict(common)
        m["x"] = np.ascontiguousarray(xs)
        in_maps.append(m)
    return in_maps


def _get_nc():
    if "nc" not in _CACHE:
        _CACHE["nc"] = _build_bass()
    return _CACHE["nc"]


def run(inputs, trace=False):
    from concourse.bass_utils import run_bass_kernel_spmd

    nc = _get_nc()
    in_maps = _prep_host(inputs)
    res = run_bass_kernel_spmd(
        nc, in_maps, core_ids=list(range(NCORES)), trace=trace)
    out = np.stack(
        [np.asarray(r["out"], np.float32).reshape(BL, C, N)
         for r in res.results], axis=0,
    ).reshape(B, C, F, F)
    return out, res


def kernel(**inputs):
    out, _ = run(inputs, trace=False)
    return out


if __name__ == "__main__":
    import reference

    ins = {k: np.asarray(v) for k, v in reference.setup_inputs().items()}
    exp = np.asarray(reference.reference(**ins))
    got = kernel(**ins)
    rel = np.linalg.norm(got - exp) / np.linalg.norm(exp)
    print("max abs err:", np.abs(got - exp).max(), "rel err:", rel)
